# revision 3
# baseline (speedup 1.0000x reference)
"""LIF spike (vanilla) Trainium2 kernel — time-bit-packed u8 output.

Reference recurrence over leading time dim T (per element):
    u_t = TAU * u_{t-1} * (1 - o_{t-1}) + x_t
    o_t = (u_t - VTH > 0) ? 1.0 : 0.0

Device-side structure per (chunk, t):
    S1 (DVE):  u_t = TAU * select(u_{t-1} <= VTH, u_{t-1}, 0) + x_t
        -- ONE custom DVE op (LIF_GATED_DECAY_ADD_ANT), registered below.
           select(u < nextafter(VTH), u, 0) == u * (u <= VTH) exactly, and
           TAU = 0.5 is a power of two so TAU*u is exact: u_t matches the
           fp32 jax reference bit-for-bit.
    S2 (ACT):  s_t = Sign(u_t - VTH) in {-1, 0, +1}, bf16
    S3 (DVE):  pair_k = s_{2k} + 0.5 * s_{2k+1}  (scalar_tensor_tensor,
               bf16-exact: values in {0, +-0.5, +-1, +-1.5})
    S4 (PE):   p += diag(2^(7-2k)) @ pair_k, k = 0..3, PSUM f32 (exact:
               2^(7-2k)*pair_k = 2^(7-2k) s_{2k} + 2^(6-2k) s_{2k+1})

The pair trick halves PE column count (the HAM-throttled PE ran at
~0.55 us per N=512 matmul and was the binding compute engine at 88 us
busy when packing one timestep per matmul).

After k=3, p = sum_t s_t * 2^(T-1-t) in [-255, 255]; one ACT copy maps
it to u8 via (p + 255)/2 (scale=0.5, bias=127.5 — exact for odd-integer
p) and one DMA per chunk stores it: output traffic is 1 MiB per core.
Host decode: unpackbits, bit (T-1-t) = spike_t.

Stream layout (profile-driven):
  - x is chunk-major on the host ([cnt, P, T, fi]); body chunks load as
    two half-chunk DMAs (t 0-3, t 4-7) so the recurrence starts before
    the full chunk lands and x slots recycle mid-chain.
  - the qSyncDynamicHW ring carries ONLY x loads, in program order —
    nothing on it ever waits on compute. (v1's o-store triggers between
    x triggers on the one sync sequencer stalled the stream every chunk.)
  - weights ride qActDynamicHW (scalar engine), output stores ride SWDGE
    (gpsimd, otherwise idle).
  - tapered classes (2048x3, 1024, 512x2); the final chunk loads per-t
    so only ~4 us of chain+store trail the last HBM byte.

Sharding: pure data parallel over batch dim B=64 -> 8 cores x 8 batches.
Per core: 32 MiB in (f32) + 1 MiB out (u8) HBM traffic.
"""

import numpy as np

T = 8
B = 64
C = 128
H = 32
W = 32
NCORES = 8
BS = B // NCORES            # batches per core
N = BS * C * H * W          # 1,048,576 elements per time step per core
P = 128                     # SBUF partitions
FTOT = N // P               # 8192 free-dim elements per partition per t
# tapered tail: (dram tensor name, count, tile free-dim)
CHUNK_CLASSES = (
    ("x2048", 3, 2048),
    ("x1024", 1, 1024),
    ("x512", 2, 512),
)
assert sum(n * f for _, n, f in CHUNK_CLASSES) == FTOT
TAU = 0.5
VTH = 0.99999
VTH_PLUS = float(np.nextafter(np.float32(VTH), np.float32(np.inf)))
PSUM_BANK_F = 512           # f32 elements per partition per PSUM bank
NPAIR = T // 2


def _digit_weights():
    """[P, NPAIR*P] bf16, partition-major: w[p, k*P+q] = 2^(7-2k) * (p==q)."""
    import ml_dtypes

    w = np.zeros((NPAIR, P, P), np.float32)
    for k in range(NPAIR):
        w[k] = np.eye(P, dtype=np.float32) * float(2 ** (T - 1 - 2 * k))
    return np.ascontiguousarray(w.transpose(1, 0, 2).reshape(P, NPAIR * P)).astype(
        ml_dtypes.bfloat16
    )


def _register_lif_op():
    """Register the fused LIF decay custom DVE op (idempotent).

    out = select(in0 < s0, in0, 0) * s1 + in1
    """
    from concourse import dve_ops
    from concourse.dve_spec import C0, C1, Spec, Src0, Src1, Zero, select
    from concourse.dve_spec import _has_src1, lower
    from concourse.dve_uop import DveOpSpec

    name = "LIF_GATED_DECAY_ADD_ANT"
    for op in dve_ops.OPS:
        if op.name == name:
            return op
    spec = Spec(
        body=select(Src0 < C0, Src0, Zero) * C1 + Src1,
        reference=lambda in0, in1, s0, s1, imm2: (
            np.where(in0 < s0, in0, np.float32(0.0)).astype(np.float32)
            * np.float32(s1)
            + in1
        ).astype(np.float32),
    )
    row = dve_ops._CUSTOM_DVE_ROW_BASE + len(dve_ops.OPS)
    assert row < 0x20, "custom-DVE opcode rows exhausted"
    shas = {}
    for ver in ("v3", "v4"):
        tmp = DveOpSpec(
            name=name, opcode=row, uops=lower(spec, ver=ver),
            rd1_en=_has_src1(spec),
        )
        shas[ver] = tmp.sha(ver)
    op = dve_ops.DveOp(name, spec, subdim=False, uops_sha=shas)
    dve_ops.OPS.append(op)
    dve_ops.CUSTOM_DVE_SPECS[name] = spec
    dve_ops._SUB_OPCODE_FOR_NAME[name] = row
    return op


def _build(nt=T):
    import concourse.bacc as bacc
    import concourse.bass as bass
    import concourse.mybir as mybir
    import concourse.tile as tile

    lif_op = _register_lif_op()

    f32 = mybir.dt.float32
    bf16 = mybir.dt.bfloat16
    u8 = mybir.dt.uint8
    act = mybir.ActivationFunctionType
    alu = mybir.AluOpType
    nc = bacc.Bacc("TRN2", target_bir_lowering=False, enable_partition_id=False)
    xts_dram = {
        name: nc.dram_tensor(name, [cnt, P, nt, fi], f32, kind="ExternalInput")
        for name, cnt, fi in CHUNK_CLASSES
    }
    ots_dram = {
        name: nc.dram_tensor("o" + name[1:], [cnt, P, fi], u8,
                             kind="ExternalOutput")
        for name, cnt, fi in CHUNK_CLASSES
    }
    w = nc.dram_tensor("w", [P, NPAIR * P], bf16, kind="ExternalInput")
    chunks = [
        (name, i, fi)
        for name, cnt, fi in CHUNK_CLASSES
        for i in range(cnt)
    ]
    with tile.TileContext(nc) as tc:
        with (
            tc.tile_pool(name="const", bufs=1) as constp,
            tc.tile_pool(name="xp", bufs=3) as xp,
            tc.tile_pool(name="xtp", bufs=9) as xtp,
            tc.tile_pool(name="up", bufs=3) as up,
            tc.tile_pool(name="sp", bufs=3) as sp,
            tc.tile_pool(name="prp", bufs=5) as prp,
            tc.tile_pool(name="op", bufs=2) as op_,
            tc.tile_pool(name="pp", bufs=2, space=bass.MemorySpace.PSUM) as pp,
        ):
            nvth = constp.tile([P, 1], f32)
            nc.vector.memset(nvth[:], -VTH)
            # weight load on the ACT (scalar) HWDGE ring so the sync ring
            # carries nothing but the x stream.
            wsb = constp.tile([P, NPAIR, P], bf16)
            nc.scalar.dma_start(wsb[:, :, :], w[:])
            for ci, (name, i, fi) in enumerate(chunks):
                xd, od = xts_dram[name], ots_dram[name]
                last = ci == len(chunks) - 1
                if not last:
                    # two half-chunk DMAs: t 0-3 and t 4-7
                    xh0 = xp.tile([P, nt // 2, fi], f32, name="xh")
                    nc.sync.dma_start(xh0[:, :, :], xd[i, :, 0 : nt // 2])
                    xh1 = xp.tile([P, nt // 2, fi], f32, name="xh")
                    nc.sync.dma_start(xh1[:, :, :], xd[i, :, nt // 2 : nt])
                    xat = lambda t: (xh0 if t < nt // 2 else xh1)[:, t % (nt // 2), :]
                else:
                    # final chunk: per-t loads so the tail chain overlaps
                    # the last transfers
                    xts = []
                    for t in range(nt):
                        xt = xtp.tile([P, fi], f32, name="xt")
                        nc.sync.dma_start(xt[:], xd[i, :, t])
                        xts.append(xt)
                    xat = lambda t: xts[t][:]
                p = pp.tile([P, fi], f32, name="p")
                u = None
                se = None  # even-t sign tile awaiting its pair partner
                for t in range(nt):
                    if t == 0:
                        u = xat(0)
                    else:
                        un = up.tile([P, fi], f32, name="un")
                        nc.vector._custom_dve(
                            lif_op, out=un[:], in0=u, in1=xat(t),
                            s0=VTH_PLUS, s1=TAU,
                        )
                        u = un[:]
                    st = sp.tile([P, fi], bf16, name="st")
                    nc.scalar.activation(
                        st[:], u, act.Sign, bias=nvth[:], scale=1.0,
                    )
                    if t % 2 == 0:
                        se = st
                        continue
                    k = t // 2
                    pr = prp.tile([P, fi], bf16, name="pr")
                    nc.vector.scalar_tensor_tensor(
                        pr[:], st[:], 0.5, se[:], alu.mult, alu.add,
                    )
                    # one Matmult may only target a single PSUM bank
                    # (512 f32 per partition): split across banks.
                    for j in range(0, fi, PSUM_BANK_F):
                        sl = slice(j, min(j + PSUM_BANK_F, fi))
                        nc.tensor.matmul(
                            p[:, sl], wsb[:, k, :], pr[:, sl],
                            start=(k == 0), stop=(k == NPAIR - 1),
                        )
                ot = op_.tile([P, fi], u8, name="ot")
                # u8 spike byte: (p + 255) / 2, exact for odd-integer p
                nc.scalar.activation(
                    ot[:], p[:], act.Copy, bias=127.5, scale=0.5,
                )
                # store via SWDGE (gpsimd) — off both HWDGE rings, so a
                # store waiting on compute never stalls the x stream.
                nc.gpsimd.dma_start(od[i], ot[:])
    nc.finalize()
    return nc


def _in_maps(x):
    wdig = _digit_weights()
    in_maps = []
    for c in range(NCORES):
        s = np.ascontiguousarray(x[:, c * BS : (c + 1) * BS]).reshape(T, N)
        m = {"w": wdig}
        base = 0
        for name, cnt, fi in CHUNK_CLASSES:
            seg = s[:, base : base + cnt * P * fi]
            # chunk-major, partition-major: [cnt, P, T, fi]
            m[name] = np.ascontiguousarray(
                seg.reshape(T, cnt, P, fi).transpose(1, 2, 0, 3)
            )
            base += cnt * P * fi
        in_maps.append(m)
    return in_maps


def kernel(x):
    x = np.ascontiguousarray(np.asarray(x, dtype=np.float32))
    assert x.shape == (T, B, C, H, W), x.shape
    from concourse.bass_utils import run_bass_kernel_spmd

    nc = _build()
    res = run_bass_kernel_spmd(nc, _in_maps(x), core_ids=list(range(NCORES)))
    out = np.empty((T, B, C, H, W), np.float32)
    for i, r in enumerate(res.results):
        out[:, i * BS : (i + 1) * BS] = _decode(r)
    return out


def _decode(r):
    """Per-core result dict -> f32 spike train [T, BS, C, H, W].

    byte = (p + 255)/2 with p = sum_t s_t * 2^(T-1-t), s_t in {-1,+1}:
    bit (T-1-t) = spike_t.
    """
    s = np.concatenate(
        [np.asarray(r["o" + name[1:]]).reshape(-1) for name, _, _ in CHUNK_CLASSES]
    )                                                          # [N] u8
    bits = np.unpackbits(s[:, None], axis=1, bitorder="big")   # [N, T]
    return bits.T.astype(np.float32).reshape(T, BS, C, H, W)


# revision 4
# speedup vs baseline: 1.2210x; 1.2210x over previous
"""LIF spike (vanilla) Trainium2 kernel — time-bit-packed u8 output.

Reference recurrence over leading time dim T (per element):
    u_t = TAU * u_{t-1} * (1 - o_{t-1}) + x_t
    o_t = (u_t - VTH > 0) ? 1.0 : 0.0

Device-side structure per (chunk, t):
    S1 (DVE):  u_t = TAU * select(u_{t-1} <= VTH, u_{t-1}, 0) + x_t
        -- ONE custom DVE op (LIF_GATED_DECAY_ADD_ANT), registered below.
           select(u < nextafter(VTH), u, 0) == u * (u <= VTH) exactly, and
           TAU = 0.5 is a power of two so TAU*u is exact: u_t matches the
           fp32 jax reference bit-for-bit.
    S2 (ACT):  s_t = Sign(u_t - VTH) in {-1, 0, +1}, bf16
    S3 (PE):   p += diag(2^(T-1-t)) @ s_t, accumulated in PSUM (f32, exact)

(PE packs one timestep per matmul. A pair-packing variant that halved PE
columns was tried and reverted: the DVE scalar_tensor_tensor it needed
runs at 1x — no 2x bf16 uop — and pushed DVE to 127 us, onto the
stream-critical chain. PE, by contrast, sits off the x-slot release
chain, so its HAM-throttled 88 us rides inside the stream window as
long as sign tiles are buffered deep enough — sp bufs=8.)

After k=3, p = sum_t s_t * 2^(T-1-t) in [-255, 255]; one ACT copy maps
it to u8 via (p + 255)/2 (scale=0.5, bias=127.5 — exact for odd-integer
p) and one DMA per chunk stores it: output traffic is 1 MiB per core.
Host decode: unpackbits, bit (T-1-t) = spike_t.

Stream layout (profile-driven):
  - x is chunk-major on the host ([cnt, P, T, fi]); body chunks load as
    two half-chunk DMAs (t 0-3, t 4-7) so the recurrence starts before
    the full chunk lands and x slots recycle mid-chain.
  - the qSyncDynamicHW ring carries ONLY x loads, in program order —
    nothing on it ever waits on compute. (v1's o-store triggers between
    x triggers on the one sync sequencer stalled the stream every chunk.)
  - weights ride qActDynamicHW (scalar engine), output stores ride SWDGE
    (gpsimd, otherwise idle).
  - tapered classes (2048x3, 1024, 512x2); the final chunk loads per-t
    so only ~4 us of chain+store trail the last HBM byte.

Sharding: pure data parallel over batch dim B=64 -> 8 cores x 8 batches.
Per core: 32 MiB in (f32) + 1 MiB out (u8) HBM traffic.
"""

import numpy as np

T = 8
B = 64
C = 128
H = 32
W = 32
NCORES = 8
BS = B // NCORES            # batches per core
N = BS * C * H * W          # 1,048,576 elements per time step per core
P = 128                     # SBUF partitions
FTOT = N // P               # 8192 free-dim elements per partition per t
# tapered tail: (dram tensor name, count, tile free-dim)
CHUNK_CLASSES = (
    ("x2048", 3, 2048),
    ("x1024", 1, 1024),
    ("x512", 2, 512),
)
assert sum(n * f for _, n, f in CHUNK_CLASSES) == FTOT
TAU = 0.5
VTH = 0.99999
VTH_PLUS = float(np.nextafter(np.float32(VTH), np.float32(np.inf)))
PSUM_BANK_F = 512           # f32 elements per partition per PSUM bank
NPAIR = T // 2


def _digit_weights():
    """[P, T*P] bf16, partition-major: w[p, t*P+q] = 2^(T-1-t) * (p==q)."""
    import ml_dtypes

    w = np.zeros((T, P, P), np.float32)
    for t in range(T):
        w[t] = np.eye(P, dtype=np.float32) * float(2 ** (T - 1 - t))
    return np.ascontiguousarray(w.transpose(1, 0, 2).reshape(P, T * P)).astype(
        ml_dtypes.bfloat16
    )


def _register_lif_op():
    """Register the fused LIF decay custom DVE op (idempotent).

    out = select(in0 < s0, in0, 0) * s1 + in1
    """
    from concourse import dve_ops
    from concourse.dve_spec import C0, C1, Spec, Src0, Src1, Zero, select
    from concourse.dve_spec import _has_src1, lower
    from concourse.dve_uop import DveOpSpec

    name = "LIF_GATED_DECAY_ADD_ANT"
    for op in dve_ops.OPS:
        if op.name == name:
            return op
    spec = Spec(
        body=select(Src0 < C0, Src0, Zero) * C1 + Src1,
        reference=lambda in0, in1, s0, s1, imm2: (
            np.where(in0 < s0, in0, np.float32(0.0)).astype(np.float32)
            * np.float32(s1)
            + in1
        ).astype(np.float32),
    )
    row = dve_ops._CUSTOM_DVE_ROW_BASE + len(dve_ops.OPS)
    assert row < 0x20, "custom-DVE opcode rows exhausted"
    shas = {}
    for ver in ("v3", "v4"):
        tmp = DveOpSpec(
            name=name, opcode=row, uops=lower(spec, ver=ver),
            rd1_en=_has_src1(spec),
        )
        shas[ver] = tmp.sha(ver)
    op = dve_ops.DveOp(name, spec, subdim=False, uops_sha=shas)
    dve_ops.OPS.append(op)
    dve_ops.CUSTOM_DVE_SPECS[name] = spec
    dve_ops._SUB_OPCODE_FOR_NAME[name] = row
    return op


def _build(nt=T):
    import concourse.bacc as bacc
    import concourse.bass as bass
    import concourse.mybir as mybir
    import concourse.tile as tile

    lif_op = _register_lif_op()

    f32 = mybir.dt.float32
    bf16 = mybir.dt.bfloat16
    u8 = mybir.dt.uint8
    act = mybir.ActivationFunctionType
    alu = mybir.AluOpType
    nc = bacc.Bacc("TRN2", target_bir_lowering=False, enable_partition_id=False)
    xts_dram = {
        name: nc.dram_tensor(name, [cnt, P, nt, fi], f32, kind="ExternalInput")
        for name, cnt, fi in CHUNK_CLASSES
    }
    ots_dram = {
        name: nc.dram_tensor("o" + name[1:], [cnt, P, fi], u8,
                             kind="ExternalOutput")
        for name, cnt, fi in CHUNK_CLASSES
    }
    w = nc.dram_tensor("w", [P, nt * P], bf16, kind="ExternalInput")
    chunks = [
        (name, i, fi)
        for name, cnt, fi in CHUNK_CLASSES
        for i in range(cnt)
    ]
    with tile.TileContext(nc) as tc:
        with (
            tc.tile_pool(name="const", bufs=1) as constp,
            tc.tile_pool(name="xp", bufs=3) as xp,
            tc.tile_pool(name="xtp", bufs=9) as xtp,
            tc.tile_pool(name="up", bufs=3) as up,
            tc.tile_pool(name="sp", bufs=8) as sp,
            tc.tile_pool(name="op", bufs=2) as op_,
            tc.tile_pool(name="pp", bufs=2, space=bass.MemorySpace.PSUM) as pp,
        ):
            nvth = constp.tile([P, 1], f32)
            nc.vector.memset(nvth[:], -VTH)
            # weight load on the ACT (scalar) HWDGE ring so the sync ring
            # carries nothing but the x stream.
            wsb = constp.tile([P, nt, P], bf16)
            nc.scalar.dma_start(wsb[:, :, :], w[:])
            for ci, (name, i, fi) in enumerate(chunks):
                xd, od = xts_dram[name], ots_dram[name]
                last = ci == len(chunks) - 1
                if not last:
                    # two half-chunk DMAs: t 0-3 and t 4-7
                    xh0 = xp.tile([P, nt // 2, fi], f32, name="xh")
                    nc.sync.dma_start(xh0[:, :, :], xd[i, :, 0 : nt // 2])
                    xh1 = xp.tile([P, nt // 2, fi], f32, name="xh")
                    nc.sync.dma_start(xh1[:, :, :], xd[i, :, nt // 2 : nt])
                    xat = lambda t: (xh0 if t < nt // 2 else xh1)[:, t % (nt // 2), :]
                else:
                    # final chunk: per-t loads so the tail chain overlaps
                    # the last transfers
                    xts = []
                    for t in range(nt):
                        xt = xtp.tile([P, fi], f32, name="xt")
                        nc.sync.dma_start(xt[:], xd[i, :, t])
                        xts.append(xt)
                    xat = lambda t: xts[t][:]
                p = pp.tile([P, fi], f32, name="p")
                u = None
                for t in range(nt):
                    if t == 0:
                        u = xat(0)
                    else:
                        un = up.tile([P, fi], f32, name="un")
                        nc.vector._custom_dve(
                            lif_op, out=un[:], in0=u, in1=xat(t),
                            s0=VTH_PLUS, s1=TAU,
                        )
                        u = un[:]
                    st = sp.tile([P, fi], bf16, name="st")
                    nc.scalar.activation(
                        st[:], u, act.Sign, bias=nvth[:], scale=1.0,
                    )
                    # one Matmult may only target a single PSUM bank
                    # (512 f32 per partition): split across banks.
                    for j in range(0, fi, PSUM_BANK_F):
                        sl = slice(j, min(j + PSUM_BANK_F, fi))
                        nc.tensor.matmul(
                            p[:, sl], wsb[:, t, :], st[:, sl],
                            start=(t == 0), stop=(t == nt - 1),
                        )
                ot = op_.tile([P, fi], u8, name="ot")
                # u8 spike byte: (p + 255) / 2, exact for odd-integer p
                nc.scalar.activation(
                    ot[:], p[:], act.Copy, bias=127.5, scale=0.5,
                )
                # store via SWDGE (gpsimd) — off both HWDGE rings, so a
                # store waiting on compute never stalls the x stream.
                nc.gpsimd.dma_start(od[i], ot[:])
    nc.finalize()
    return nc


def _in_maps(x):
    wdig = _digit_weights()
    in_maps = []
    for c in range(NCORES):
        s = np.ascontiguousarray(x[:, c * BS : (c + 1) * BS]).reshape(T, N)
        m = {"w": wdig}
        base = 0
        for name, cnt, fi in CHUNK_CLASSES:
            seg = s[:, base : base + cnt * P * fi]
            # chunk-major, partition-major: [cnt, P, T, fi]
            m[name] = np.ascontiguousarray(
                seg.reshape(T, cnt, P, fi).transpose(1, 2, 0, 3)
            )
            base += cnt * P * fi
        in_maps.append(m)
    return in_maps


def kernel(x):
    x = np.ascontiguousarray(np.asarray(x, dtype=np.float32))
    assert x.shape == (T, B, C, H, W), x.shape
    from concourse.bass_utils import run_bass_kernel_spmd

    nc = _build()
    res = run_bass_kernel_spmd(nc, _in_maps(x), core_ids=list(range(NCORES)))
    out = np.empty((T, B, C, H, W), np.float32)
    for i, r in enumerate(res.results):
        out[:, i * BS : (i + 1) * BS] = _decode(r)
    return out


def _decode(r):
    """Per-core result dict -> f32 spike train [T, BS, C, H, W].

    byte = (p + 255)/2 with p = sum_t s_t * 2^(T-1-t), s_t in {-1,+1}:
    bit (T-1-t) = spike_t.
    """
    s = np.concatenate(
        [np.asarray(r["o" + name[1:]]).reshape(-1) for name, _, _ in CHUNK_CLASSES]
    )                                                          # [N] u8
    bits = np.unpackbits(s[:, None], axis=1, bitorder="big")   # [N, T]
    return bits.T.astype(np.float32).reshape(T, BS, C, H, W)


# revision 5
# speedup vs baseline: 1.3963x; 1.1436x over previous
"""LIF spike (vanilla) Trainium2 kernel — time-bit-packed u8 output.

Reference recurrence over leading time dim T (per element):
    u_t = TAU * u_{t-1} * (1 - o_{t-1}) + x_t
    o_t = (u_t - VTH > 0) ? 1.0 : 0.0

Device-side structure per (chunk, t):
    S1 (DVE):  u_t = TAU * select(u_{t-1} <= VTH, u_{t-1}, 0) + x_t
        -- ONE custom DVE op (LIF_GATED_DECAY_ADD_ANT), registered below.
           select(u < nextafter(VTH), u, 0) == u * (u <= VTH) exactly, and
           TAU = 0.5 is a power of two so TAU*u is exact: u_t matches the
           fp32 jax reference bit-for-bit.
    S2 (ACT):  s_t = Sign(u_t - VTH) in {-1, 0, +1}, bf16
    S3 (PE):   p += diag(2^(T-1-t)) @ s_t, accumulated in PSUM (f32, exact)

(PE packs one timestep per matmul. A pair-packing variant that halved PE
columns was tried and reverted: the DVE scalar_tensor_tensor it needed
runs at 1x — no 2x bf16 uop — and pushed DVE to 127 us, onto the
stream-critical chain. PE, by contrast, sits off the x-slot release
chain, so its HAM-throttled 88 us rides inside the stream window as
long as sign tiles are buffered deep enough — sp bufs=8.)

After k=3, p = sum_t s_t * 2^(T-1-t) in [-255, 255]; one ACT copy maps
it to u8 via (p + 255)/2 (scale=0.5, bias=127.5 — exact for odd-integer
p) and one DMA per chunk stores it: output traffic is 1 MiB per core.
Host decode: unpackbits, bit (T-1-t) = spike_t.

Stream layout (profile-driven):
  - x is chunk-major on the host ([cnt, P, T, fi]); body chunks load as
    two half-chunk DMAs (t 0-3, t 4-7) so the recurrence starts before
    the full chunk lands and x slots recycle mid-chain.
  - the qSyncDynamicHW ring carries ONLY x loads, in program order —
    nothing on it ever waits on compute. (v1's o-store triggers between
    x triggers on the one sync sequencer stalled the stream every chunk.)
  - weights ride qActDynamicHW (scalar engine), output stores ride SWDGE
    (gpsimd, otherwise idle).
  - tapered classes (2048x3, 1024, 512x2); the final chunk loads per-t
    so only ~4 us of chain+store trail the last HBM byte.

Sharding: pure data parallel over batch dim B=64 -> 8 cores x 8 batches.
Per core: 32 MiB in (f32) + 1 MiB out (u8) HBM traffic.
"""

import numpy as np

T = 8
B = 64
C = 128
H = 32
W = 32
NCORES = 8
BS = B // NCORES            # batches per core
N = BS * C * H * W          # 1,048,576 elements per time step per core
P = 128                     # SBUF partitions
FTOT = N // P               # 8192 free-dim elements per partition per t
# tapered tail: (dram tensor name, count, tile free-dim)
CHUNK_CLASSES = (
    ("x2048", 3, 2048),
    ("x1024", 1, 1024),
    ("x512", 2, 512),
)
assert sum(n * f for _, n, f in CHUNK_CLASSES) == FTOT
TAU = 0.5
VTH = 0.99999
VTH_PLUS = float(np.nextafter(np.float32(VTH), np.float32(np.inf)))
PSUM_BANK_F = 512           # f32 elements per partition per PSUM bank
TAIL_NAME = "x512"        # per-t-loaded tail class
NT_PE = T - 1             # timesteps packed on PE; t=T-1 joins via DVE STT


def _digit_weights():
    """[P, NT_PE*P] bf16, partition-major: w[p, t*P+q] = 2^(T-1-t)*(p==q)."""
    import ml_dtypes

    w = np.zeros((NT_PE, P, P), np.float32)
    for t in range(NT_PE):
        w[t] = np.eye(P, dtype=np.float32) * float(2 ** (T - 1 - t))
    return np.ascontiguousarray(w.transpose(1, 0, 2).reshape(P, NT_PE * P)).astype(
        ml_dtypes.bfloat16
    )


def _register_lif_op():
    """Register the fused LIF decay custom DVE op (idempotent).

    out = select(in0 < s0, in0, 0) * s1 + in1
    """
    from concourse import dve_ops
    from concourse.dve_spec import C0, C1, Spec, Src0, Src1, Zero, select
    from concourse.dve_spec import _has_src1, lower
    from concourse.dve_uop import DveOpSpec

    name = "LIF_GATED_DECAY_ADD_ANT"
    for op in dve_ops.OPS:
        if op.name == name:
            return op
    spec = Spec(
        body=select(Src0 < C0, Src0, Zero) * C1 + Src1,
        reference=lambda in0, in1, s0, s1, imm2: (
            np.where(in0 < s0, in0, np.float32(0.0)).astype(np.float32)
            * np.float32(s1)
            + in1
        ).astype(np.float32),
    )
    row = dve_ops._CUSTOM_DVE_ROW_BASE + len(dve_ops.OPS)
    assert row < 0x20, "custom-DVE opcode rows exhausted"
    shas = {}
    for ver in ("v3", "v4"):
        tmp = DveOpSpec(
            name=name, opcode=row, uops=lower(spec, ver=ver),
            rd1_en=_has_src1(spec),
        )
        shas[ver] = tmp.sha(ver)
    op = dve_ops.DveOp(name, spec, subdim=False, uops_sha=shas)
    dve_ops.OPS.append(op)
    dve_ops.CUSTOM_DVE_SPECS[name] = spec
    dve_ops._SUB_OPCODE_FOR_NAME[name] = row
    return op


def _build(nt=T):
    import concourse.bacc as bacc
    import concourse.bass as bass
    import concourse.mybir as mybir
    import concourse.tile as tile

    lif_op = _register_lif_op()

    f32 = mybir.dt.float32
    bf16 = mybir.dt.bfloat16
    u8 = mybir.dt.uint8
    act = mybir.ActivationFunctionType
    alu = mybir.AluOpType
    nc = bacc.Bacc("TRN2", target_bir_lowering=False, enable_partition_id=False)
    # body classes: [cnt, 2, P, (nt//2)*fi] — each half-chunk DMA is fully
    # contiguous per partition (strided halves measured ~25% slower).
    # tail class (x512): [cnt, nt, P, fi] — per-t loads, each contiguous.
    xts_dram = {
        name: (
            nc.dram_tensor(name, [cnt, nt, P, fi], f32, kind="ExternalInput")
            if name == TAIL_NAME
            else nc.dram_tensor(
                name, [cnt, 2, P, (nt // 2) * fi], f32, kind="ExternalInput"
            )
        )
        for name, cnt, fi in CHUNK_CLASSES
    }
    ots_dram = {
        name: nc.dram_tensor("o" + name[1:], [cnt, P, fi], u8,
                             kind="ExternalOutput")
        for name, cnt, fi in CHUNK_CLASSES
    }
    w = nc.dram_tensor("w", [P, NT_PE * P], bf16, kind="ExternalInput")
    chunks = [
        (name, i, fi)
        for name, cnt, fi in CHUNK_CLASSES
        for i in range(cnt)
    ]
    with tile.TileContext(nc) as tc:
        with (
            tc.tile_pool(name="const", bufs=1) as constp,
            tc.tile_pool(name="xp", bufs=3) as xp,
            tc.tile_pool(name="xtp", bufs=9) as xtp,
            tc.tile_pool(name="up", bufs=3) as up,
            tc.tile_pool(name="sp", bufs=8) as sp,
            tc.tile_pool(name="wp", bufs=2) as wp,
            tc.tile_pool(name="op", bufs=2) as op_,
            tc.tile_pool(name="pp", bufs=2, space=bass.MemorySpace.PSUM) as pp,
        ):
            nvth = constp.tile([P, 1], f32)
            nc.vector.memset(nvth[:], -VTH)
            # weight load on the ACT (scalar) HWDGE ring so the sync ring
            # carries nothing but the x stream.
            wsb = constp.tile([P, NT_PE, P], bf16)
            nc.scalar.dma_start(wsb[:, :, :], w[:])
            for ci, (name, i, fi) in enumerate(chunks):
                xd, od = xts_dram[name], ots_dram[name]
                if name != TAIL_NAME:
                    # two half-chunk DMAs: t 0-3 and t 4-7, each contiguous
                    xh0 = xp.tile([P, nt // 2, fi], f32, name="xh")
                    nc.sync.dma_start(xh0[:, :, :], xd[i, 0])
                    xh1 = xp.tile([P, nt // 2, fi], f32, name="xh")
                    nc.sync.dma_start(xh1[:, :, :], xd[i, 1])
                    xat = lambda t: (xh0 if t < nt // 2 else xh1)[:, t % (nt // 2), :]
                else:
                    # tail chunks: per-t contiguous loads so the tail chain
                    # overlaps the last transfers
                    xts = []
                    for t in range(nt):
                        xt = xtp.tile([P, fi], f32, name="xt")
                        nc.sync.dma_start(xt[:], xd[i, t])
                        xts.append(xt)
                    xat = lambda t: xts[t][:]
                p = pp.tile([P, fi], f32, name="p")
                u = None
                for t in range(nt):
                    if t == 0:
                        u = xat(0)
                    else:
                        un = up.tile([P, fi], f32, name="un")
                        nc.vector._custom_dve(
                            lif_op, out=un[:], in0=u, in1=xat(t),
                            s0=VTH_PLUS, s1=TAU,
                        )
                        u = un[:]
                    st = sp.tile([P, fi], bf16, name="st")
                    nc.scalar.activation(
                        st[:], u, act.Sign, bias=nvth[:], scale=1.0,
                    )
                    if t == nt - 1:
                        # last timestep joins via DVE below — PE packs t<7
                        s_last = st
                        continue
                    # one Matmult may only target a single PSUM bank
                    # (512 f32 per partition): split across banks.
                    for j in range(0, fi, PSUM_BANK_F):
                        sl = slice(j, min(j + PSUM_BANK_F, fi))
                        nc.tensor.matmul(
                            p[:, sl], wsb[:, t, :], st[:, sl],
                            start=(t == 0), stop=(t == nt - 2),
                        )
                # w16 = p + 255 + s_7 (DVE STT, psum read) — rebalances one
                # timestep of pack work from the HAM-throttled PE onto DVE.
                w16 = wp.tile([P, fi], f32, name="w16")
                nc.vector.scalar_tensor_tensor(
                    w16[:], p[:], 255.0, s_last[:], alu.add, alu.add,
                )
                ot = op_.tile([P, fi], u8, name="ot")
                # u8 spike byte: (p + 255 + s_7)/2, exact (odd integer)
                nc.scalar.activation(
                    ot[:], w16[:], act.Copy, bias=0.0, scale=0.5,
                )
                # store via SWDGE (gpsimd) — off both HWDGE rings, so a
                # store waiting on compute never stalls the x stream.
                nc.gpsimd.dma_start(od[i], ot[:])
    nc.finalize()
    return nc


def _in_maps(x):
    wdig = _digit_weights()
    in_maps = []
    for c in range(NCORES):
        s = np.ascontiguousarray(x[:, c * BS : (c + 1) * BS]).reshape(T, N)
        m = {"w": wdig}
        base = 0
        for name, cnt, fi in CHUNK_CLASSES:
            seg = s[:, base : base + cnt * P * fi].reshape(T, cnt, P, fi)
            if name == TAIL_NAME:
                # per-t-major: [cnt, T, P, fi]
                m[name] = np.ascontiguousarray(seg.transpose(1, 0, 2, 3))
            else:
                # chunk-major, half-major, partition-major:
                # [cnt, 2, P, (T//2)*fi]
                m[name] = np.ascontiguousarray(
                    seg.reshape(2, T // 2, cnt, P, fi).transpose(2, 0, 3, 1, 4)
                ).reshape(cnt, 2, P, (T // 2) * fi)
            base += cnt * P * fi
        in_maps.append(m)
    return in_maps


def kernel(x):
    x = np.ascontiguousarray(np.asarray(x, dtype=np.float32))
    assert x.shape == (T, B, C, H, W), x.shape
    from concourse.bass_utils import run_bass_kernel_spmd

    nc = _build()
    res = run_bass_kernel_spmd(nc, _in_maps(x), core_ids=list(range(NCORES)))
    out = np.empty((T, B, C, H, W), np.float32)
    for i, r in enumerate(res.results):
        out[:, i * BS : (i + 1) * BS] = _decode(r)
    return out


def _decode(r):
    """Per-core result dict -> f32 spike train [T, BS, C, H, W].

    byte = (p + 255)/2 with p = sum_t s_t * 2^(T-1-t), s_t in {-1,+1}:
    bit (T-1-t) = spike_t.
    """
    s = np.concatenate(
        [np.asarray(r["o" + name[1:]]).reshape(-1) for name, _, _ in CHUNK_CLASSES]
    )                                                          # [N] u8
    bits = np.unpackbits(s[:, None], axis=1, bitorder="big")   # [N, T]
    return bits.T.astype(np.float32).reshape(T, BS, C, H, W)


# revision 6
# speedup vs baseline: 1.4049x; 1.0062x over previous
"""LIF spike (vanilla) Trainium2 kernel — time-bit-packed u8 output.

Reference recurrence over leading time dim T (per element):
    u_t = TAU * u_{t-1} * (1 - o_{t-1}) + x_t
    o_t = (u_t - VTH > 0) ? 1.0 : 0.0

Device-side structure per (chunk, t):
    S1 (DVE):  u_t = TAU * select(u_{t-1} <= VTH, u_{t-1}, 0) + x_t
        -- ONE custom DVE op (LIF_GATED_DECAY_ADD_ANT), registered below.
           select(u < nextafter(VTH), u, 0) == u * (u <= VTH) exactly, and
           TAU = 0.5 is a power of two so TAU*u is exact: u_t matches the
           fp32 jax reference bit-for-bit.
    S2 (ACT):  s_t = Sign(u_t - VTH) in {-1, 0, +1}, bf16
    S3 (PE):   p += diag(2^(T-1-t)) @ s_t, accumulated in PSUM (f32, exact)

(PE packs one timestep per matmul. A pair-packing variant that halved PE
columns was tried and reverted: the DVE scalar_tensor_tensor it needed
runs at 1x — no 2x bf16 uop — and pushed DVE to 127 us, onto the
stream-critical chain. PE, by contrast, sits off the x-slot release
chain, so its HAM-throttled 88 us rides inside the stream window as
long as sign tiles are buffered deep enough — sp bufs=8.)

After k=3, p = sum_t s_t * 2^(T-1-t) in [-255, 255]; one ACT copy maps
it to u8 via (p + 255)/2 (scale=0.5, bias=127.5 — exact for odd-integer
p) and one DMA per chunk stores it: output traffic is 1 MiB per core.
Host decode: unpackbits, bit (T-1-t) = spike_t.

Stream layout (profile-driven):
  - x is chunk-major on the host ([cnt, P, T, fi]); body chunks load as
    two half-chunk DMAs (t 0-3, t 4-7) so the recurrence starts before
    the full chunk lands and x slots recycle mid-chain.
  - the qSyncDynamicHW ring carries ONLY x loads, in program order —
    nothing on it ever waits on compute. (v1's o-store triggers between
    x triggers on the one sync sequencer stalled the stream every chunk.)
  - weights ride qActDynamicHW (scalar engine), output stores ride SWDGE
    (gpsimd, otherwise idle).
  - tapered classes (2048x3, 1024, 512x2); the final chunk loads per-t
    so only ~4 us of chain+store trail the last HBM byte.

Sharding: pure data parallel over batch dim B=64 -> 8 cores x 8 batches.
Per core: 32 MiB in (f32) + 1 MiB out (u8) HBM traffic.
"""

import numpy as np

T = 8
B = 64
C = 128
H = 32
W = 32
NCORES = 8
BS = B // NCORES            # batches per core
N = BS * C * H * W          # 1,048,576 elements per time step per core
P = 128                     # SBUF partitions
FTOT = N // P               # 8192 free-dim elements per partition per t
# tapered tail: (dram tensor name, count, tile free-dim)
CHUNK_CLASSES = (
    ("x2048", 3, 2048),
    ("x1024", 1, 1024),
    ("x512b", 1, 512),
    ("x512t", 1, 512),
)
assert sum(n * f for _, n, f in CHUNK_CLASSES) == FTOT
TAU = 0.5
VTH = 0.99999
VTH_PLUS = float(np.nextafter(np.float32(VTH), np.float32(np.inf)))
PSUM_BANK_F = 512           # f32 elements per partition per PSUM bank
TAIL_NAME = "x512t"       # per-t-loaded tail class (the final chunk)
NT_PE = T - 1             # timesteps packed on PE; t=T-1 joins via DVE STT


def _digit_weights():
    """[P, NT_PE*P] bf16, partition-major: w[p, t*P+q] = 2^(T-1-t)*(p==q)."""
    import ml_dtypes

    w = np.zeros((NT_PE, P, P), np.float32)
    for t in range(NT_PE):
        w[t] = np.eye(P, dtype=np.float32) * float(2 ** (T - 1 - t))
    return np.ascontiguousarray(w.transpose(1, 0, 2).reshape(P, NT_PE * P)).astype(
        ml_dtypes.bfloat16
    )


def _register_lif_op():
    """Register the fused LIF decay custom DVE op (idempotent).

    out = select(in0 < s0, in0, 0) * s1 + in1
    """
    from concourse import dve_ops
    from concourse.dve_spec import C0, C1, Spec, Src0, Src1, Zero, select
    from concourse.dve_spec import _has_src1, lower
    from concourse.dve_uop import DveOpSpec

    name = "LIF_GATED_DECAY_ADD_ANT"
    for op in dve_ops.OPS:
        if op.name == name:
            return op
    spec = Spec(
        body=select(Src0 < C0, Src0, Zero) * C1 + Src1,
        reference=lambda in0, in1, s0, s1, imm2: (
            np.where(in0 < s0, in0, np.float32(0.0)).astype(np.float32)
            * np.float32(s1)
            + in1
        ).astype(np.float32),
    )
    row = dve_ops._CUSTOM_DVE_ROW_BASE + len(dve_ops.OPS)
    assert row < 0x20, "custom-DVE opcode rows exhausted"
    shas = {}
    for ver in ("v3", "v4"):
        tmp = DveOpSpec(
            name=name, opcode=row, uops=lower(spec, ver=ver),
            rd1_en=_has_src1(spec),
        )
        shas[ver] = tmp.sha(ver)
    op = dve_ops.DveOp(name, spec, subdim=False, uops_sha=shas)
    dve_ops.OPS.append(op)
    dve_ops.CUSTOM_DVE_SPECS[name] = spec
    dve_ops._SUB_OPCODE_FOR_NAME[name] = row
    return op


def _build(nt=T):
    import concourse.bacc as bacc
    import concourse.bass as bass
    import concourse.mybir as mybir
    import concourse.tile as tile

    lif_op = _register_lif_op()

    f32 = mybir.dt.float32
    bf16 = mybir.dt.bfloat16
    u8 = mybir.dt.uint8
    u16 = mybir.dt.uint16
    act = mybir.ActivationFunctionType
    alu = mybir.AluOpType
    nc = bacc.Bacc("TRN2", target_bir_lowering=False, enable_partition_id=False)
    # body classes: [cnt, 2, P, (nt//2)*fi] — each half-chunk DMA is fully
    # contiguous per partition (strided halves measured ~25% slower).
    # tail class (x512): [cnt, nt, P, fi] — per-t loads, each contiguous.
    xts_dram = {
        name: (
            nc.dram_tensor(name, [cnt, nt, P, fi], f32, kind="ExternalInput")
            if name == TAIL_NAME
            else nc.dram_tensor(
                name, [cnt, 2, P, (nt // 2) * fi], f32, kind="ExternalInput"
            )
        )
        for name, cnt, fi in CHUNK_CLASSES
    }
    ots_dram = {
        name: nc.dram_tensor("o" + name[1:], [cnt, P, fi], u16,
                             kind="ExternalOutput")
        for name, cnt, fi in CHUNK_CLASSES
    }
    w = nc.dram_tensor("w", [P, NT_PE * P], bf16, kind="ExternalInput")
    chunks = [
        (name, i, fi)
        for name, cnt, fi in CHUNK_CLASSES
        for i in range(cnt)
    ]
    with tile.TileContext(nc) as tc:
        with (
            tc.tile_pool(name="const", bufs=1) as constp,
            tc.tile_pool(name="xp", bufs=3) as xp,
            tc.tile_pool(name="xtp", bufs=9) as xtp,
            tc.tile_pool(name="up", bufs=4) as up,
            tc.tile_pool(name="sp", bufs=8) as sp,
            tc.tile_pool(name="wp", bufs=3) as wp,
            tc.tile_pool(name="pp", bufs=2, space=bass.MemorySpace.PSUM) as pp,
        ):
            nvth = constp.tile([P, 1], f32)
            nc.vector.memset(nvth[:], -VTH)
            # weight load on the ACT (scalar) HWDGE ring so the sync ring
            # carries nothing but the x stream.
            wsb = constp.tile([P, NT_PE, P], bf16)
            nc.scalar.dma_start(wsb[:, :, :], w[:])
            for ci, (name, i, fi) in enumerate(chunks):
                xd, od = xts_dram[name], ots_dram[name]
                if name != TAIL_NAME:
                    # two half-chunk DMAs: t 0-3 and t 4-7, each contiguous
                    xh0 = xp.tile([P, nt // 2, fi], f32, name="xh")
                    nc.sync.dma_start(xh0[:, :, :], xd[i, 0])
                    xh1 = xp.tile([P, nt // 2, fi], f32, name="xh")
                    nc.sync.dma_start(xh1[:, :, :], xd[i, 1])
                    xat = lambda t: (xh0 if t < nt // 2 else xh1)[:, t % (nt // 2), :]
                else:
                    # tail chunks: per-t contiguous loads so the tail chain
                    # overlaps the last transfers
                    xts = []
                    for t in range(nt):
                        xt = xtp.tile([P, fi], f32, name="xt")
                        nc.sync.dma_start(xt[:], xd[i, t])
                        xts.append(xt)
                    xat = lambda t: xts[t][:]
                p = pp.tile([P, fi], f32, name="p")
                u = None
                for t in range(nt):
                    if t == 0:
                        u = xat(0)
                    else:
                        un = up.tile([P, fi], f32, name="un")
                        nc.vector._custom_dve(
                            lif_op, out=un[:], in0=u, in1=xat(t),
                            s0=VTH_PLUS, s1=TAU,
                        )
                        u = un[:]
                    st = sp.tile([P, fi], bf16, name="st")
                    nc.scalar.activation(
                        st[:], u, act.Sign, bias=nvth[:], scale=1.0,
                    )
                    if t == nt - 1:
                        # last timestep joins via DVE below — PE packs t<7
                        s_last = st
                        continue
                    # one Matmult may only target a single PSUM bank
                    # (512 f32 per partition): split across banks.
                    for j in range(0, fi, PSUM_BANK_F):
                        sl = slice(j, min(j + PSUM_BANK_F, fi))
                        nc.tensor.matmul(
                            p[:, sl], wsb[:, t, :], st[:, sl],
                            start=(t == 0), stop=(t == nt - 2),
                        )
                # w16 = p + 255 + s_7 (DVE STT, psum read, u16 out) —
                # rebalances one timestep of pack work off the
                # HAM-throttled PE AND replaces the ACT psum->u8 copy
                # (host decodes byte = w16 >> 1; w16 is even: p has only
                # even coefficients and 255 + s_7 is even).
                w16 = wp.tile([P, fi], u16, name="w16")
                nc.vector.scalar_tensor_tensor(
                    w16[:], p[:], 255.0, s_last[:], alu.add, alu.add,
                )
                # store via SWDGE (gpsimd) — off both HWDGE rings, so a
                # store waiting on compute never stalls the x stream.
                nc.gpsimd.dma_start(od[i], w16[:])
    nc.finalize()
    return nc


def _in_maps(x):
    wdig = _digit_weights()
    in_maps = []
    for c in range(NCORES):
        s = np.ascontiguousarray(x[:, c * BS : (c + 1) * BS]).reshape(T, N)
        m = {"w": wdig}
        base = 0
        for name, cnt, fi in CHUNK_CLASSES:
            seg = s[:, base : base + cnt * P * fi].reshape(T, cnt, P, fi)
            if name == TAIL_NAME:
                # per-t-major: [cnt, T, P, fi]
                m[name] = np.ascontiguousarray(seg.transpose(1, 0, 2, 3))
            else:
                # chunk-major, half-major, partition-major:
                # [cnt, 2, P, (T//2)*fi]
                m[name] = np.ascontiguousarray(
                    seg.reshape(2, T // 2, cnt, P, fi).transpose(2, 0, 3, 1, 4)
                ).reshape(cnt, 2, P, (T // 2) * fi)
            base += cnt * P * fi
        in_maps.append(m)
    return in_maps


def kernel(x):
    x = np.ascontiguousarray(np.asarray(x, dtype=np.float32))
    assert x.shape == (T, B, C, H, W), x.shape
    from concourse.bass_utils import run_bass_kernel_spmd

    nc = _build()
    res = run_bass_kernel_spmd(nc, _in_maps(x), core_ids=list(range(NCORES)))
    out = np.empty((T, B, C, H, W), np.float32)
    for i, r in enumerate(res.results):
        out[:, i * BS : (i + 1) * BS] = _decode(r)
    return out


def _decode(r):
    """Per-core result dict -> f32 spike train [T, BS, C, H, W].

    w16 = p(t<7) + 255 + s_7; byte = w16 >> 1 = (sum_t s_t 2^(T-1-t) + 255)/2,
    s_t in {-1,+1}: bit (T-1-t) = spike_t.
    """
    v = np.concatenate(
        [np.asarray(r["o" + name[1:]]).reshape(-1) for name, _, _ in CHUNK_CLASSES]
    )                                                          # [N] u16
    s = (v >> 1).astype(np.uint8)                              # (p+255+s7)/2
    bits = np.unpackbits(s[:, None], axis=1, bitorder="big")   # [N, T]
    return bits.T.astype(np.float32).reshape(T, BS, C, H, W)
